# revision 14
# baseline (speedup 1.0000x reference)
"""Supervised-contrastive loss on 8 TRN2 NeuronCores — v6 (symmetric bands).

Math (matches the reference exactly):
    s_ij  = cosine similarity of feature rows i, j
    E_ij  = exp(s_ij / tau)
    neg_i = sum_j E_ij * (1 - mask_ij)        (mask = same-class, incl. diag)
    loss  = sum over i and same-class j != i of [ln(E_ij + neg_i) - s_ij/tau] / p_i
            ------------------------------------------------------------------
                                 sum_i p_i

v6 key change vs v5: exploit E_ij == E_ji.  Rows are sorted by class on
the host; the NxN matrix is viewed as 32x32 blocks of 128x128.  Row block
r computes only the circulant band of 17 column blocks starting at its
diagonal (d = 0..16).  Every unordered block pair {r, s} with distance
d = (s-r) mod 32 in {1..15} is computed exactly once (by the lower-d
side); d == 16 pairs are computed by BOTH sides but consumed rowsum-only;
d == 0 (diagonal) once.  Per row the device produces:
  - rowsum_i = sum of E over the row's own band (ACT fused accumulator),
  - colsum_j = sum over the band's d in {1..15} columns of E (ones-vector
    matmul over a DVE-accumulated bf16 E buffer) -> credited to the
    transposed rows on the host,
  - a 256-wide diagonal slab of raw S (covers all same-class pairs
    (i, j<=i+127); host reconstructs both triangles by symmetry).
This halves both the ACT exp stream (the v5 bottleneck: 16.8M -> 8.9M
exps) and the fp8 DoubleRow GEMM.

Per core: 4 row tiles x band 2176 = 8 chunks of [128, 1088].  PSUM: 2x3
banks for S chunks + 2x1 bank for the colsum sweep.  The moving operand
is pre-rotated per core so row tile `it`'s band is local fn cols
[128*it, 128*it + 2176); one SPMD program for all cores.

Host postprocessing (unmeasured) reassembles rsE = rowsum + scattered
colsum, gathers class-window S values from the slabs (using symmetry for
the j < i half), and computes the final scalar in f64.
"""

import numpy as np
import ml_dtypes

TAU = 0.1
N, D = 4096, 512
NCORES = 8
ROWS = N // NCORES          # 512 rows per core
ITILES = ROWS // 128        # 4 partition tiles per core
BAND = 2176                 # 17 blocks: d = 0..16
CHUNKW = BAND // 2          # 1088
NCH = 2                     # chunks per row tile
FNW = 2560                  # local fn cols needed: [0, 384 + 2176)
CSW = 960                   # colsum cols per chunk (d 1..15 half)
AW = 3 * 128 + 2 * CSW      # 2304: colsum accumulator width
SLAB = 256                  # raw-S slab width per row tile
GSCALE = 16.0               # per-operand pre-scale before fp8 quantization
SSCALE = GSCALE * GSCALE    # S' = SSCALE * S
NDUMMY = 6

_CACHE = {}


def _build_nc():
    import concourse.tile as tile
    import concourse.mybir as mybir
    from concourse import bacc

    dt = mybir.dt
    AF = mybir.ActivationFunctionType

    KP = 2                              # fp8 DoubleRow: 2 contraction passes
    KS = 2                              # k-subtiles packed per pass

    nc = bacc.Bacc(None)
    # DoubleRow-ready layout: [p, kp*KS + s, x]; local col x = global
    # (512*core + x) mod N
    fnT = nc.declare_dram_parameter("fnT", [128, KP * KS, FNW], dt.float8e4,
                                    isOutput=False)
    rse_out = nc.declare_dram_parameter(
        "rse_out", [128, ITILES * NCH], dt.float32, isOutput=True)
    slab_out = nc.declare_dram_parameter(
        "slab_out", [128, ITILES * SLAB], dt.float32, isOutput=True)
    csum_out = nc.declare_dram_parameter(
        "csum_out", [128, AW // 128], dt.float32, isOutput=True)

    with tile.TileContext(nc) as tc:
        with (
            tc.tile_pool(name="persist", bufs=1) as persist,
            tc.tile_pool(name="psum", bufs=2, space="PSUM") as psum,
            tc.tile_pool(name="cps", bufs=1, space="PSUM") as cps,
            tc.tile_pool(name="ebuf", bufs=3) as ebuf,
            tc.tile_pool(name="outp", bufs=1) as outp,
        ):
            # ---- operand loads: per (ksub, col-piece), contiguous dest runs
            # (128 descriptors each), on the two HWDGE queues only (gpsimd's
            # SWDGE path measured ~4x slower).  Piece 0 = [0, 1472): all
            # four c0 chunks + stationary weights; piece 1 = the c1 tail.
            # scalar gets just two issues so ACT_TABLE_LOAD + the exp
            # stream start early.
            H0 = 1472
            fn_sb = persist.tile([128, KP * KS, FNW], dt.float8e4, tag="fnt")
            with tc.high_priority():
                for k, eng, h0, h1 in (
                    (0, nc.sync, 0, H0),
                    (2, nc.scalar, 0, H0),
                    (1, nc.sync, 0, H0),
                    (3, nc.scalar, 0, H0),
                    (0, nc.sync, H0, FNW),
                    (1, nc.sync, H0, FNW),
                    (2, nc.sync, H0, FNW),
                    (3, nc.sync, H0, FNW),
                ):
                    eng.dma_start(
                        fn_sb[:, k:k + 1, h0:h1],
                        fnT[:, k:k + 1, h0:h1])

            rse_sb = outp.tile([128, ITILES * NCH], dt.float32, tag="rse")
            slab_sb = outp.tile([128, ITILES * SLAB], dt.float32, tag="slab")
            acc_sb = outp.tile([128, AW], dt.bfloat16, tag="acc")
            ones_sb = outp.tile([128, 1], dt.bfloat16, tag="ones")
            nc.vector.memset(acc_sb[:], 0.0)
            nc.vector.memset(ones_sb[:], 1.0)

            dumm = slab_sb.bitcast(dt.bfloat16)       # [128, 2*ITILES*SLAB]

            def gemm_chunk(S, it, c):
                b0 = 128 * it + CHUNKW * c
                for kp in range(KP):
                    for f, w in ((0, 512), (512, 512), (1024, 64)):
                        nc.tensor.matmul(
                            S[:, f:f + w],
                            fn_sb[:, kp * KS:(kp + 1) * KS,
                                  128 * it:128 * it + 128],
                            fn_sb[:, kp * KS:(kp + 1) * KS,
                                  b0 + f:b0 + f + w],
                            start=(kp == 0),
                            stop=(kp == KP - 1),
                            perf_mode=mybir.MatmulPerfMode.DoubleRow,
                        )

            for c in range(NCH):
                for it in range(ITILES):
                    S = psum.tile([128, CHUNKW], dt.float32, tag="S")
                    if it == 0 and c == 0:
                        # PE p-state priming on garbage SBUF while the
                        # operand DMAs are in flight; borrows S's banks.
                        for _ in range(NDUMMY - 2):
                            nc.tensor.matmul(
                                S[:, 0:512], dumm[:, 0:128], dumm[:, 128:640],
                                start=True, stop=True,
                                skip_group_check=True,
                            )
                        for _ in range(12):
                            nc.tensor.matmul(
                                S[:, 0:128], dumm[:, 0:128], dumm[:, 128:256],
                                start=True, stop=True,
                                skip_group_check=True,
                            )
                    gemm_chunk(S, it, c)
                    # exp first: keeps the DVE slab copy off the ACT stream's
                    # critical path (same-tile readers chain in issue order).
                    E = ebuf.tile([128, CHUNKW], dt.bfloat16, tag="E")
                    nc.scalar.activation(
                        E[:], S[:], AF.Exp,
                        scale=1.0 / (SSCALE * TAU),
                        accum_out=rse_sb[:, it * NCH + c:it * NCH + c + 1],
                    )
                    if c == 0:
                        # raw-S slab: band cols [0, 256) hold every
                        # same-class pair (i, j) with i <= j <= i+127
                        nc.vector.tensor_copy(
                            slab_sb[:, it * SLAB:(it + 1) * SLAB],
                            S[:, 0:SLAB],
                        )
                        if it == ITILES - 1:
                            nc.gpsimd.dma_start(slab_out[:], slab_sb[:])
                        # colsum region: band cols [128, 1088) -> A[128*it ..)
                        a0 = 128 * it
                        nc.vector.tensor_add(
                            acc_sb[:, a0:a0 + CSW],
                            acc_sb[:, a0:a0 + CSW],
                            E[:, 128:128 + CSW],
                        )
                    else:
                        # colsum region: band cols [1088, 2048) (d 16 block
                        # [2048, 2176) excluded: rowsum-only on both sides)
                        a0 = 128 * it + CSW
                        nc.vector.tensor_add(
                            acc_sb[:, a0:a0 + CSW],
                            acc_sb[:, a0:a0 + CSW],
                            E[:, 0:CSW],
                        )

            # ---- colsum sweep: A^T @ ones, transposed so each 128-col block
            # of A yields a [128, 1] PSUM column (no slow [1, n] copies) ----
            NB = AW // 128
            CPT = cps.tile([128, NB], dt.float32, tag="CPT")
            csum_sb = outp.tile([128, NB], dt.float32, tag="csb")
            for b in range(NB):
                nc.tensor.matmul(
                    CPT[:, b:b + 1],
                    acc_sb[:, 128 * b:128 * (b + 1)],
                    ones_sb[:, 0:1],
                    start=True, stop=True,
                )
            nc.vector.tensor_copy(csum_sb[:], CPT[:])
            nc.sync.dma_start(csum_out[:], csum_sb[:])

            # rsE flush from the sync queue (idle after the input loads).
            nc.sync.dma_start(rse_out[:], rse_sb[:])

    nc.finalize()
    return nc


def _get_nc():
    if "nc" not in _CACHE:
        _CACHE["nc"] = _build_nc()
    return _CACHE["nc"]


def _host_prep(features, targets):
    np_dt = ml_dtypes.float8_e4m3
    KP, KS = 2, 2
    f = np.asarray(features, np.float32)
    t = np.asarray(targets).astype(np.int64)
    norm = np.sqrt((f.astype(np.float64) ** 2).sum(1))
    rnorm = np.where(norm > 0, 1.0 / np.maximum(norm, 1e-300), 0.0)
    fn = (f * rnorm[:, None].astype(np.float32)).astype(np.float32)

    order = np.argsort(t, kind="stable")
    fns = fn[order]
    fq = (fns * GSCALE).astype(np_dt)
    fqT = np.ascontiguousarray(fq.T)            # [D, N]

    def dr_layout(a):
        # [D, X] -> [128, KP*KS, X] with row d = (kp*KS + s)*128 + p
        X = a.shape[1]
        return np.ascontiguousarray(
            a.reshape(KP, KS, 128, X).transpose(2, 0, 1, 3)
             .reshape(128, KP * KS, X))

    in_maps = []
    for c in range(NCORES):
        cols = (512 * c + np.arange(FNW)) % N
        in_maps.append({"fnT": dr_layout(np.ascontiguousarray(fqT[:, cols]))})
    return (t, order), in_maps


def _host_post(aux, per_core_outs):
    t, order = aux
    ts = t[order]

    rse = np.zeros(N, np.float64)
    slab = np.empty((N, SLAB), np.float64)
    for c, out in enumerate(per_core_outs):
        ra = np.asarray(out["rse_out"], np.float64)      # [128, ITILES*NCH]
        sa = np.asarray(out["slab_out"], np.float64)     # [128, ITILES*SLAB]
        for it in range(ITILES):
            rows = slice(c * ROWS + it * 128, c * ROWS + (it + 1) * 128)
            rse[rows] = ra[:, it * NCH:(it + 1) * NCH].sum(1)
            slab[rows] = sa[:, it * SLAB:(it + 1) * SLAB]
    for c, out in enumerate(per_core_outs):
        # csum_out[m, b] = colsum of A col 128*b + m
        cs = np.asarray(out["csum_out"], np.float64).T.reshape(-1)  # [AW]
        # A col a covers global col (512c + 128 + a) mod N
        np.add.at(rse, (512 * c + 128 + np.arange(AW)) % N, cs)
    slab /= SSCALE

    # class windows in sorted space
    classes, first_idx, counts = np.unique(
        ts, return_index=True, return_counts=True)
    rank = np.searchsorted(classes, ts)
    o_row = first_idx[rank]                  # window start (global col)
    n_row = counts[rank].astype(np.int64)    # p_i
    assert n_row.max() <= 128, f"class size {n_row.max()} > 128"

    W = int(n_row.max())
    ii = np.arange(N)[:, None]
    jj = o_row[:, None] + np.arange(W)[None, :]
    valid = np.arange(W)[None, :] < n_row[:, None]
    jc = np.minimum(jj, N - 1)
    # S_ij: j >= i from row i's slab, j < i from row j's slab (symmetry)
    lo = np.minimum(ii, jc)
    hi = np.maximum(ii, jc)
    col = hi - 128 * (lo >> 7)
    sv = slab[lo, np.minimum(col, SLAB - 1)]
    z = sv / TAU
    Ew = np.exp(z) * valid
    possum = Ew.sum(1)
    neg = rse - possum

    m2 = valid.copy()
    m2[np.arange(N), np.arange(N) - o_row] = False   # drop diagonal
    lnsum = (np.log(Ew + neg[:, None], where=m2, out=np.zeros_like(Ew))
             * m2).sum(1)
    bsum = (z * m2).sum(1)
    numer = (lnsum - bsum) / n_row
    loss = numer.sum() / n_row.sum()
    return np.float32(loss)


def _run(in_maps, trace=False):
    from concourse.bass_utils import run_bass_kernel_spmd
    nc = _get_nc()
    res = run_bass_kernel_spmd(
        nc, in_maps, core_ids=list(range(NCORES)), trace=trace,
    )
    return res


def kernel(features, targets):
    aux, in_maps = _host_prep(features, targets)
    res = _run(in_maps, trace=False)
    return _host_post(aux, res.results)


# revision 16
# speedup vs baseline: 1.0055x; 1.0055x over previous
"""Supervised-contrastive loss on 8 TRN2 NeuronCores — v6 (symmetric bands).

Math (matches the reference exactly):
    s_ij  = cosine similarity of feature rows i, j
    E_ij  = exp(s_ij / tau)
    neg_i = sum_j E_ij * (1 - mask_ij)        (mask = same-class, incl. diag)
    loss  = sum over i and same-class j != i of [ln(E_ij + neg_i) - s_ij/tau] / p_i
            ------------------------------------------------------------------
                                 sum_i p_i

v6 key change vs v5: exploit E_ij == E_ji.  Rows are sorted by class on
the host; the NxN matrix is viewed as 32x32 blocks of 128x128.  Row block
r computes only the circulant band of 17 column blocks starting at its
diagonal (d = 0..16).  Every unordered block pair {r, s} with distance
d = (s-r) mod 32 in {1..15} is computed exactly once (by the lower-d
side); d == 16 pairs are computed by BOTH sides but consumed rowsum-only;
d == 0 (diagonal) once.  Per row the device produces:
  - rowsum_i = sum of E over the row's own band (ACT fused accumulator),
  - colsum_j = sum over the band's d in {1..15} columns of E (ones-vector
    matmul over a DVE-accumulated bf16 E buffer) -> credited to the
    transposed rows on the host,
  - a 256-wide diagonal slab of raw S (covers all same-class pairs
    (i, j<=i+127); host reconstructs both triangles by symmetry).
This halves both the ACT exp stream (the v5 bottleneck: 16.8M -> 8.9M
exps) and the fp8 DoubleRow GEMM.

Per core: 4 row tiles x band 2176 = 8 chunks of [128, 1088].  PSUM: 2x3
banks for S chunks + 2x1 bank for the colsum sweep.  The moving operand
is pre-rotated per core so row tile `it`'s band is local fn cols
[128*it, 128*it + 2176); one SPMD program for all cores.

Host postprocessing (unmeasured) reassembles rsE = rowsum + scattered
colsum, gathers class-window S values from the slabs (using symmetry for
the j < i half), and computes the final scalar in f64.
"""

import numpy as np
import ml_dtypes

TAU = 0.1
N, D = 4096, 512
NCORES = 8
ROWS = N // NCORES          # 512 rows per core
ITILES = ROWS // 128        # 4 partition tiles per core
BAND = 2176                 # 17 blocks: d = 0..16
CHUNKW = BAND // 2          # 1088
NCH = 2                     # chunks per row tile
FNW = 2560                  # local fn cols needed: [0, 384 + 2176)
CSW = 960                   # colsum cols per chunk (d 1..15 half)
AW = 3 * 128 + 2 * CSW      # 2304: colsum accumulator width
SLAB = 256                  # raw-S slab width per row tile
GSCALE = 16.0               # per-operand pre-scale before fp8 quantization
SSCALE = GSCALE * GSCALE    # S' = SSCALE * S
NDUMMY = 6

_CACHE = {}


def _build_nc():
    import concourse.tile as tile
    import concourse.mybir as mybir
    from concourse import bacc

    dt = mybir.dt
    AF = mybir.ActivationFunctionType

    KP = 2                              # fp8 DoubleRow: 2 contraction passes
    KS = 2                              # k-subtiles packed per pass

    nc = bacc.Bacc(None)
    # DoubleRow-ready layout: [p, kp*KS + s, x]; local col x = global
    # (512*core + x) mod N
    fnT = nc.declare_dram_parameter("fnT", [128, KP * KS, FNW], dt.float8e4,
                                    isOutput=False)
    rse_out = nc.declare_dram_parameter(
        "rse_out", [128, ITILES * NCH], dt.float32, isOutput=True)
    slab_out = nc.declare_dram_parameter(
        "slab_out", [128, ITILES * SLAB], dt.float32, isOutput=True)
    csum_out = nc.declare_dram_parameter(
        "csum_out", [128, AW // 128], dt.float32, isOutput=True)

    with tile.TileContext(nc) as tc:
        with (
            tc.tile_pool(name="persist", bufs=1) as persist,
            tc.tile_pool(name="psum", bufs=2, space="PSUM") as psum,
            tc.tile_pool(name="cps", bufs=1, space="PSUM") as cps,
            tc.tile_pool(name="ebuf", bufs=3) as ebuf,
            tc.tile_pool(name="outp", bufs=1) as outp,
        ):
            # ---- operand loads: per (ksub, col-piece), contiguous dest runs
            # (128 descriptors each), on the two HWDGE queues only (gpsimd's
            # SWDGE path measured ~4x slower).  Piece 0 = [0, 1472): all
            # four c0 chunks + stationary weights; piece 1 = the c1 tail.
            # scalar gets just two issues so ACT_TABLE_LOAD + the exp
            # stream start early.
            # piece boundaries: [0,1088) = chunk (0,0) + stationary;
            # [1088,1472) completes all c0 chunks; [1472,2560) the c1 tail.
            # Tile deps gate on whole DMA pieces, so finer pieces unblock
            # earlier chunks sooner.
            fn_sb = persist.tile([128, KP * KS, FNW], dt.float8e4, tag="fnt")
            with tc.high_priority():
                for k, eng, h0, h1 in (
                    (0, nc.sync, 0, 1088),
                    (2, nc.scalar, 0, 1088),
                    (1, nc.sync, 0, 1088),
                    (3, nc.scalar, 0, 1088),
                    (0, nc.sync, 1088, 1472),
                    (2, nc.scalar, 1088, 1472),
                    (1, nc.sync, 1088, 1472),
                    (3, nc.scalar, 1088, 1472),
                    (0, nc.sync, 1472, FNW),
                    (1, nc.sync, 1472, FNW),
                    (2, nc.gpsimd, 1472, FNW),
                    (3, nc.gpsimd, 1472, FNW),
                ):
                    eng.dma_start(
                        fn_sb[:, k:k + 1, h0:h1],
                        fnT[:, k:k + 1, h0:h1])

            rse_sb = outp.tile([128, ITILES * NCH], dt.float32, tag="rse")
            slab_sb = outp.tile([128, ITILES * SLAB], dt.float32, tag="slab")
            acc_sb = outp.tile([128, AW], dt.bfloat16, tag="acc")
            ones_sb = outp.tile([128, 1], dt.bfloat16, tag="ones")
            nc.vector.memset(acc_sb[:], 0.0)
            nc.vector.memset(ones_sb[:], 1.0)

            dumm = slab_sb.bitcast(dt.bfloat16)       # [128, 2*ITILES*SLAB]

            def gemm_chunk(S, it, c):
                b0 = 128 * it + CHUNKW * c
                for kp in range(KP):
                    for f, w in ((0, 512), (512, 512), (1024, 64)):
                        nc.tensor.matmul(
                            S[:, f:f + w],
                            fn_sb[:, kp * KS:(kp + 1) * KS,
                                  128 * it:128 * it + 128],
                            fn_sb[:, kp * KS:(kp + 1) * KS,
                                  b0 + f:b0 + f + w],
                            start=(kp == 0),
                            stop=(kp == KP - 1),
                            perf_mode=mybir.MatmulPerfMode.DoubleRow,
                        )

            for c in range(NCH):
                for it in range(ITILES):
                    S = psum.tile([128, CHUNKW], dt.float32, tag="S")
                    if it == 0 and c == 0:
                        # PE p-state priming on garbage SBUF while the
                        # operand DMAs are in flight; borrows S's banks.
                        for _ in range(3):
                            nc.tensor.matmul(
                                S[:, 0:512], dumm[:, 0:128], dumm[:, 128:640],
                                start=True, stop=True,
                                skip_group_check=True,
                            )
                        for _ in range(6):
                            nc.tensor.matmul(
                                S[:, 0:128], dumm[:, 0:128], dumm[:, 128:256],
                                start=True, stop=True,
                                skip_group_check=True,
                            )
                    gemm_chunk(S, it, c)
                    # exp first: keeps the DVE slab copy off the ACT stream's
                    # critical path (same-tile readers chain in issue order).
                    E = ebuf.tile([128, CHUNKW], dt.bfloat16, tag="E")
                    nc.scalar.activation(
                        E[:], S[:], AF.Exp,
                        scale=1.0 / (SSCALE * TAU),
                        accum_out=rse_sb[:, it * NCH + c:it * NCH + c + 1],
                    )
                    if c == 0:
                        # raw-S slab: band cols [0, 256) hold every
                        # same-class pair (i, j) with i <= j <= i+127
                        nc.vector.tensor_copy(
                            slab_sb[:, it * SLAB:(it + 1) * SLAB],
                            S[:, 0:SLAB],
                        )
                        if it == ITILES - 1:
                            nc.gpsimd.dma_start(slab_out[:], slab_sb[:])
                        # colsum region: band cols [128, 1088) -> A[128*it ..)
                        a0 = 128 * it
                        nc.vector.tensor_add(
                            acc_sb[:, a0:a0 + CSW],
                            acc_sb[:, a0:a0 + CSW],
                            E[:, 128:128 + CSW],
                        )
                    else:
                        # colsum region: band cols [1088, 2048) (d 16 block
                        # [2048, 2176) excluded: rowsum-only on both sides)
                        a0 = 128 * it + CSW
                        nc.vector.tensor_add(
                            acc_sb[:, a0:a0 + CSW],
                            acc_sb[:, a0:a0 + CSW],
                            E[:, 0:CSW],
                        )

            # ---- colsum sweep: A^T @ ones, transposed so each 128-col block
            # of A yields a [128, 1] PSUM column (no slow [1, n] copies) ----
            NB = AW // 128
            CPT = cps.tile([128, NB], dt.float32, tag="CPT")
            csum_sb = outp.tile([128, NB], dt.float32, tag="csb")
            for b in range(NB):
                nc.tensor.matmul(
                    CPT[:, b:b + 1],
                    acc_sb[:, 128 * b:128 * (b + 1)],
                    ones_sb[:, 0:1],
                    start=True, stop=True,
                )
            nc.vector.tensor_copy(csum_sb[:], CPT[:])
            nc.sync.dma_start(csum_out[:], csum_sb[:])

            # rsE flush from the sync queue (idle after the input loads).
            nc.sync.dma_start(rse_out[:], rse_sb[:])

    nc.finalize()
    return nc


def _get_nc():
    if "nc" not in _CACHE:
        _CACHE["nc"] = _build_nc()
    return _CACHE["nc"]


def _host_prep(features, targets):
    np_dt = ml_dtypes.float8_e4m3
    KP, KS = 2, 2
    f = np.asarray(features, np.float32)
    t = np.asarray(targets).astype(np.int64)
    norm = np.sqrt((f.astype(np.float64) ** 2).sum(1))
    rnorm = np.where(norm > 0, 1.0 / np.maximum(norm, 1e-300), 0.0)
    fn = (f * rnorm[:, None].astype(np.float32)).astype(np.float32)

    order = np.argsort(t, kind="stable")
    fns = fn[order]
    fq = (fns * GSCALE).astype(np_dt)
    fqT = np.ascontiguousarray(fq.T)            # [D, N]

    def dr_layout(a):
        # [D, X] -> [128, KP*KS, X] with row d = (kp*KS + s)*128 + p
        X = a.shape[1]
        return np.ascontiguousarray(
            a.reshape(KP, KS, 128, X).transpose(2, 0, 1, 3)
             .reshape(128, KP * KS, X))

    in_maps = []
    for c in range(NCORES):
        cols = (512 * c + np.arange(FNW)) % N
        in_maps.append({"fnT": dr_layout(np.ascontiguousarray(fqT[:, cols]))})
    return (t, order), in_maps


def _host_post(aux, per_core_outs):
    t, order = aux
    ts = t[order]

    rse = np.zeros(N, np.float64)
    slab = np.empty((N, SLAB), np.float64)
    for c, out in enumerate(per_core_outs):
        ra = np.asarray(out["rse_out"], np.float64)      # [128, ITILES*NCH]
        sa = np.asarray(out["slab_out"], np.float64)     # [128, ITILES*SLAB]
        for it in range(ITILES):
            rows = slice(c * ROWS + it * 128, c * ROWS + (it + 1) * 128)
            rse[rows] = ra[:, it * NCH:(it + 1) * NCH].sum(1)
            slab[rows] = sa[:, it * SLAB:(it + 1) * SLAB]
    for c, out in enumerate(per_core_outs):
        # csum_out[m, b] = colsum of A col 128*b + m
        cs = np.asarray(out["csum_out"], np.float64).T.reshape(-1)  # [AW]
        # A col a covers global col (512c + 128 + a) mod N
        np.add.at(rse, (512 * c + 128 + np.arange(AW)) % N, cs)
    slab /= SSCALE

    # class windows in sorted space
    classes, first_idx, counts = np.unique(
        ts, return_index=True, return_counts=True)
    rank = np.searchsorted(classes, ts)
    o_row = first_idx[rank]                  # window start (global col)
    n_row = counts[rank].astype(np.int64)    # p_i
    assert n_row.max() <= 128, f"class size {n_row.max()} > 128"

    W = int(n_row.max())
    ii = np.arange(N)[:, None]
    jj = o_row[:, None] + np.arange(W)[None, :]
    valid = np.arange(W)[None, :] < n_row[:, None]
    jc = np.minimum(jj, N - 1)
    # S_ij: j >= i from row i's slab, j < i from row j's slab (symmetry)
    lo = np.minimum(ii, jc)
    hi = np.maximum(ii, jc)
    col = hi - 128 * (lo >> 7)
    sv = slab[lo, np.minimum(col, SLAB - 1)]
    z = sv / TAU
    Ew = np.exp(z) * valid
    possum = Ew.sum(1)
    neg = rse - possum

    m2 = valid.copy()
    m2[np.arange(N), np.arange(N) - o_row] = False   # drop diagonal
    lnsum = (np.log(Ew + neg[:, None], where=m2, out=np.zeros_like(Ew))
             * m2).sum(1)
    bsum = (z * m2).sum(1)
    numer = (lnsum - bsum) / n_row
    loss = numer.sum() / n_row.sum()
    return np.float32(loss)


def _run(in_maps, trace=False):
    from concourse.bass_utils import run_bass_kernel_spmd
    nc = _get_nc()
    res = run_bass_kernel_spmd(
        nc, in_maps, core_ids=list(range(NCORES)), trace=trace,
    )
    return res


def kernel(features, targets):
    aux, in_maps = _host_prep(features, targets)
    res = _run(in_maps, trace=False)
    return _host_post(aux, res.results)
